# revision 1
# baseline (speedup 1.0000x reference)
"""GQA (32 q heads / 8 kv heads, RoPE, causal) Trainium2 Bass kernel.

Sharding: tensor-parallel over kv heads — core c owns kv head c and q heads
4c..4c+3 for both batches. Each core computes a partial o-projection
(its 256 attn channels x Wo columns) and the host sums the 8 partials.

Device-side structure (per core, per batch):
  * Fused QKV projection: one accumulation chain per 128-token tile produces
    [t, 384] = [4 q heads | k head | v head] with d contracted on partitions
    (host passes x pre-transposed).  float32r matmuls (1 cycle/row).
  * RoPE applied in token-partition layout with stride-2 free-dim APs
    (interleaved even/odd pairs), 6 DVE ops per tile covering all 5 heads.
  * Q/K transposed per-head via TensorE into [dh, t] (f32r), V kept natural
    [t, dh] with a ones column appended.
  * Scores computed transposed [keys, queries]; exp on ACT (no max needed:
    |scores| small by construction); causal diagonal masked by DVE multiply.
  * attn.V matmul gives attnT [dh, i] plus the softmax denominator for free
    (ones row of V); normalization via reciprocal + PE broadcast + DVE mul
    writes attnT directly into the o-projection's stationary layout [c, t].
"""

import numpy as np
from contextlib import ExitStack

import concourse.bass as bass
from concourse import bacc
import concourse.mybir as mybir
import concourse.tile as tile
from concourse.bass_utils import run_bass_kernel_spmd

B, S, D = 2, 2048, 2048
DH = 64            # head dim
G = 4              # q heads per core (= per kv head)
NCORES = 8
TT = 512           # attention i-tile
NTT = S // TT      # 4
KC = D // 128      # 16 contraction chunks
NJC = S // 128     # 16 token/key chunks of 128
F32 = mybir.dt.float32
F32R = mybir.dt.float32r
ROPE_BASE = 10000.0

_cached = {}


def build_nc():
    nc = bacc.Bacc("TRN2", target_bir_lowering=False, debug=False)
    xt = nc.declare_dram_parameter("xt", [B, D, S], F32, isOutput=False)
    wall = nc.declare_dram_parameter("wall", [D, 384], F32, isOutput=False)
    wot = nc.declare_dram_parameter("wot", [256, D], F32, isOutput=False)
    cosr = nc.declare_dram_parameter("cosr", [S, 160], F32, isOutput=False)
    sinr = nc.declare_dram_parameter("sinr", [S, 160], F32, isOutput=False)
    cmask = nc.declare_dram_parameter("cmask", [4, 128, TT], F32, isOutput=False)
    ident = nc.declare_dram_parameter("ident", [128, 128], F32, isOutput=False)
    o = nc.declare_dram_parameter("o", [B, S, D], F32, isOutput=True)

    EXP = mybir.ActivationFunctionType.Exp

    with tile.TileContext(nc) as tc, ExitStack() as ctx:
        wpool = ctx.enter_context(tc.tile_pool(name="weights", bufs=1))
        per_b = ctx.enter_context(tc.tile_pool(name="per_b", bufs=1))
        xpool = ctx.enter_context(tc.tile_pool(name="xstream", bufs=12))
        qkvpool = ctx.enter_context(tc.tile_pool(name="qkv", bufs=3))
        epool = ctx.enter_context(tc.tile_pool(name="exp", bufs=6))
        rpool = ctx.enter_context(tc.tile_pool(name="rope", bufs=2))
        opool = ctx.enter_context(tc.tile_pool(name="out", bufs=4))
        spool = ctx.enter_context(tc.tile_pool(name="small", bufs=4))
        pp_proj = ctx.enter_context(tc.tile_pool(name="pproj", bufs=1, space="PSUM"))
        pp_att = ctx.enter_context(tc.tile_pool(name="patt", bufs=2, space="PSUM"))
        pp_av = ctx.enter_context(tc.tile_pool(name="pav", bufs=1, space="PSUM"))
        pp_misc = ctx.enter_context(tc.tile_pool(name="pmisc", bufs=1, space="PSUM"))

        # ---- persistent weights/tables ----
        wall_sb = wpool.tile([128, KC, 384], F32R, tag="wall")
        wot_sb = wpool.tile([128, 2, D], F32R, tag="wot")
        cos_sb = wpool.tile([128, NJC, 160], F32, tag="cos")
        sin_sb = wpool.tile([128, NJC, 160], F32, tag="sin")
        mask_sb = wpool.tile([128, 4, TT], F32R, tag="mask")
        ident_sb = wpool.tile([128, 128], F32, tag="ident")
        ones_sb = wpool.tile([1, 64], F32R, tag="ones")
        for k in range(KC):
            nc.sync.dma_start(wall_sb[:, k, :],
                              wall[k * 128:(k + 1) * 128, :].bitcast(F32R))
        for cc in range(2):
            nc.sync.dma_start(wot_sb[:, cc, :],
                              wot[cc * 128:(cc + 1) * 128, :].bitcast(F32R))
        for j in range(NJC):
            nc.sync.dma_start(cos_sb[:, j, :], cosr[j * 128:(j + 1) * 128, :])
            nc.sync.dma_start(sin_sb[:, j, :], sinr[j * 128:(j + 1) * 128, :])
        for m in range(4):
            nc.sync.dma_start(mask_sb[:, m, :], cmask[m].bitcast(F32R))
        nc.sync.dma_start(ident_sb[:], ident[:, :])
        nc.vector.memset(ones_sb[:].bitcast(F32), 1.0)

        for b in range(B):
            qt = per_b.tile([64, G, S], F32R, tag="qt")
            kt = per_b.tile([64, S], F32R, tag="kt")
            vsb = per_b.tile([128, NJC, DH + 1], F32R, tag="vsb")
            at = per_b.tile([128, 2, S], F32R, tag="at")
            nc.vector.memset(vsb[:].bitcast(F32), 1.0)

            # ---------- fused QKV projection + rope + transposes ----------
            # Transposes for tile tt are emitted after tile tt+1's matmuls so
            # the PE never waits on the ACT-evict -> DVE-rope chain.
            def emit_tail(tt, qkv):
                tsl = slice(tt * 128, (tt + 1) * 128)
                for h in range(5):
                    ptr = pp_misc.tile([64, 128], F32, tag="misc")
                    nc.tensor.transpose(ptr[:], qkv[:, h * 64:(h + 1) * 64],
                                        ident_sb[:, :])
                    if h < G:
                        nc.vector.tensor_copy(qt[:, h, tsl], ptr[:])
                    else:
                        nc.vector.tensor_copy(kt[:, tsl], ptr[:])
                nc.vector.tensor_copy(vsb[:, tt, 0:DH], qkv[:, 320:384])

            prev = None
            for tg in range(4):             # groups of 512 tokens, 4 psum accs
                pq = [pp_proj.tile([128, 384], F32, tag=f"pq{s}",
                                   name=f"pq{s}_{b}_{tg}")
                      for s in range(4)]
                for k in range(KC):
                    xbig = xpool.tile([128, 512], F32R, tag="xt")
                    nc.sync.dma_start(
                        xbig[:],
                        xt[b, k * 128:(k + 1) * 128,
                           tg * 512:(tg + 1) * 512].bitcast(F32R))
                    for s in range(4):
                        nc.tensor.matmul(pq[s][:],
                                         xbig[:, s * 128:(s + 1) * 128],
                                         wall_sb[:, k, :],
                                         start=(k == 0), stop=(k == KC - 1))
                for s in range(4):
                    tt = tg * 4 + s
                    qkv = qkvpool.tile([128, 384], F32, tag="qkv")
                    nc.scalar.copy(qkv[:], pq[s][:])
                    # rope on q+k (cols 0:320), interleaved pairs in free dim
                    pear = qkv[:, 0:320].rearrange("p (h i two) -> p h i two",
                                                   two=2, i=32)
                    ev, od = pear[:, :, :, 0], pear[:, :, :, 1]
                    cs = cos_sb[:, tt, :].rearrange("p (h i) -> p h i", i=32)
                    sn = sin_sb[:, tt, :].rearrange("p (h i) -> p h i", i=32)
                    ec = rpool.tile([128, 5, 32], F32, tag="ec")
                    es = rpool.tile([128, 5, 32], F32, tag="es")
                    oc = rpool.tile([128, 5, 32], F32, tag="oc")
                    os_ = rpool.tile([128, 5, 32], F32, tag="os")
                    nc.vector.tensor_mul(ec[:], ev, cs)
                    nc.vector.tensor_mul(es[:], ev, sn)
                    nc.vector.tensor_mul(oc[:], od, cs)
                    nc.vector.tensor_mul(os_[:], od, sn)
                    nc.vector.tensor_sub(ev, ec[:], os_[:])
                    nc.vector.tensor_add(od, es[:], oc[:])
                    if prev is not None:
                        emit_tail(*prev)
                    prev = (tt, qkv)
            emit_tail(*prev)

            # ---------- attention ----------
            for g in range(G):
                cc, r0 = g // 2, (g % 2) * 64
                for it in range(NTT):
                    isl = slice(it * TT, (it + 1) * TT)
                    pav = pp_av.tile([65, TT], F32, tag="av")
                    njc = 4 * it + 4
                    pending = []  # attn.V pipelined two steps behind scores
                    for jc in range(njc):
                        psc = pp_att.tile([128, TT], F32, tag="sc")
                        nc.tensor.matmul(
                            psc[:], kt[:, jc * 128:(jc + 1) * 128],
                            qt[:, g, isl], start=True, stop=True)
                        esb = epool.tile([128, TT], F32R, tag="exp")
                        nc.scalar.activation(esb[:], psc[:], EXP, scale=0.125)
                        if jc >= 4 * it:  # diagonal block: causal mask
                            nc.vector.tensor_mul(esb[:], esb[:],
                                                 mask_sb[:, jc - 4 * it, :])
                        pending.append(((pav[:], vsb[:, jc, :], esb[:]),
                                        dict(start=(jc == 0),
                                             stop=(jc == njc - 1))))
                        if len(pending) > 2:
                            a = pending.pop(0)
                            nc.tensor.matmul(*a[0], **a[1])
                    for a in pending:
                        nc.tensor.matmul(*a[0], **a[1])
                    # normalize via ones-row sum: recip -> PE broadcast -> mul
                    rcp = spool.tile([1, TT], F32, tag="rcp")
                    nc.vector.reciprocal(rcp[:], pav[64:65, :])
                    avs = spool.tile([64, TT], F32, tag="avs")
                    nc.scalar.copy(avs[:], pav[0:64, :])
                    rcpr = spool.tile([1, TT], F32R, tag="rcpr")
                    nc.vector.tensor_copy(rcpr[:], rcp[:])
                    pbc = pp_misc.tile([64, TT], F32, tag="misc")
                    nc.tensor.matmul(pbc[:], ones_sb[:], rcpr[:],
                                     start=True, stop=True)
                    nc.vector.tensor_mul(at[r0:r0 + 64, cc, isl],
                                         avs[:], pbc[:])

            # ---------- o projection (partial over this core's channels) ----
            for tt in range(NJC):
                tsl = slice(tt * 128, (tt + 1) * 128)
                for nt in range(D // TT):
                    nsl = slice(nt * TT, (nt + 1) * TT)
                    po = pp_proj.tile([128, TT], F32, tag=f"pq{nt}",
                                      name=f"po{b}_{tt}_{nt}")
                    nc.tensor.matmul(po[:], at[:, 0, tsl], wot_sb[:, 0, nsl],
                                     start=True, stop=False)
                    nc.tensor.matmul(po[:], at[:, 1, tsl], wot_sb[:, 1, nsl],
                                     start=False, stop=True)
                    osb = opool.tile([128, TT], F32, tag="osb")
                    nc.vector.tensor_copy(osb[:], po[:])
                    nc.sync.dma_start(o[b, tsl, nsl], osb[:])
    nc.compile()
    return nc


def host_inputs(x, Wq, Wk, Wv, Wo):
    """Per-core input maps. Q/K weight rows permuted so each head is
    [interleaved] kept natural; rope works on interleaved pairs in the
    free dim, so NO permutation is needed here."""
    xtp = np.ascontiguousarray(np.transpose(np.asarray(x, np.float32), (0, 2, 1)))
    inv = ROPE_BASE ** (-np.arange(0, DH, 2, dtype=np.float64) / DH)
    th = np.arange(S, dtype=np.float64)[:, None] * inv[None, :]  # (S, 32)
    cosr = np.tile(np.cos(th), (1, 5)).astype(np.float32)  # (S, 160)
    sinr = np.tile(np.sin(th), (1, 5)).astype(np.float32)
    p = np.arange(128)[:, None]
    f = np.arange(TT)[None, :]
    cmask = np.stack([(p + m * 128 <= f).astype(np.float32) for m in range(4)])
    ident = np.eye(128, dtype=np.float32)
    in_maps = []
    for c in range(NCORES):
        wall = np.concatenate([Wq[256 * c:256 * (c + 1)],
                               Wk[DH * c:DH * (c + 1)],
                               Wv[DH * c:DH * (c + 1)]], axis=0)
        wall = np.ascontiguousarray(wall.T.astype(np.float32))       # (D, 384)
        wot = np.ascontiguousarray(Wo[:, 256 * c:256 * (c + 1)].T
                                   .astype(np.float32))              # (256, D)
        in_maps.append(dict(xt=xtp, wall=wall, wot=wot, cosr=cosr,
                            sinr=sinr, cmask=cmask, ident=ident))
    return in_maps


def kernel(**inputs):
    x = np.asarray(inputs["x"], dtype=np.float32)
    Wq = np.asarray(inputs["Wq"], dtype=np.float32)
    Wk = np.asarray(inputs["Wk"], dtype=np.float32)
    Wv = np.asarray(inputs["Wv"], dtype=np.float32)
    Wo = np.asarray(inputs["Wo"], dtype=np.float32)
    in_maps = host_inputs(x, Wq, Wk, Wv, Wo)
    if "nc" not in _cached:
        _cached["nc"] = build_nc()
    res = run_bass_kernel_spmd(_cached["nc"], in_maps, list(range(NCORES)))
    out = np.zeros((B, S, D), np.float64)
    for r in res.results:
        out += r["o"]
    return out.astype(np.float32)



# revision 5
# speedup vs baseline: 1.1482x; 1.1482x over previous
"""GQA (32 q heads / 8 kv heads, RoPE, causal) Trainium2 Bass kernel.

Sharding: tensor-parallel over kv heads -- core c owns kv head c and q heads
4c..4c+3 for both batches. Each core computes a partial o-projection
(its 256 attn channels x Wo columns) in fp16 and the host sums the 8 partials.

Device-side structure (per core, per batch):
  * QKV projection in fp16: stationary x chunks [d,128 tok], moving fused
    W [d, 384] = [4 q heads | k | v]; psum evicted to fp16 by DVE.
  * RoPE on DVE in token-partition layout; head dims are host-permuted to
    [32 evens | 32 odds] so the pair slices are stride-1 (DVE fast mode).
  * Q/K transposed to [dh, tok] via DMA-XBAR transposes (no PE involvement);
    K replicated to partitions 64:127 with one SBUF-to-SBUF DMA per batch so
    odd heads can use matching base partitions.
  * Scores computed [keys, q] per 128-key chunk; exp on ACT (scale=1/8, no
    max needed); causal diagonal masked by DVE multiply in fp16.
  * attn.V restructured: out [q, dh+1] psum (65 rows per 128-key chunk, half
    the PE cost of the [dh, q] form); ones column of V gives the softmax
    denominator; normalization = DVE reciprocal + per-partition scalar mul.
  * Normalized attn tiles [q, dh] are pair-packed and DMA-XBAR-transposed
    into the o-projection stationary layout [channels, tok].
  * o-projection psum evicted to fp16 by GPSIMD; o written back by
    GPSIMD-issued SWDGE DMAs (keeps the shared HWDGE free).
  * QKV matmuls of batch 1 and o-proj matmuls of batch 0 are interleaved
    into the ACT-bound attention phases so the PE never starves.
"""

import numpy as np
from collections import deque
from contextlib import ExitStack

import concourse.bass as bass
from concourse import bacc
import concourse.mybir as mybir
import concourse.tile as tile
from concourse.bass_utils import run_bass_kernel_spmd

B, S, D = 2, 2048, 2048
DH = 64            # head dim
G = 4              # q heads per core (= per kv head)
NCORES = 8
TT = 512           # attention i-tile
NTT = S // TT      # 4
KC = D // 128      # 16 contraction chunks
NJC = S // 128     # 16 token/key chunks of 128
F32 = mybir.dt.float32
F16 = mybir.dt.float16
ROPE_BASE = 10000.0

_cached = {}


def build_nc():
    nc = bacc.Bacc("TRN2", target_bir_lowering=False, debug=False)
    xt = nc.declare_dram_parameter("xt", [B, D, S], F16, isOutput=False)
    wall = nc.declare_dram_parameter("wall", [D, 384], F16, isOutput=False)
    wot = nc.declare_dram_parameter("wot", [256, D], F16, isOutput=False)
    cosr = nc.declare_dram_parameter("cosr", [S, 160], F16, isOutput=False)
    sinr = nc.declare_dram_parameter("sinr", [S, 160], F16, isOutput=False)
    cmask = nc.declare_dram_parameter("cmask", [4, 128, TT], F16, isOutput=False)
    o = nc.declare_dram_parameter("o", [B, S, D], F16, isOutput=True)

    EXP = mybir.ActivationFunctionType.Exp

    with tile.TileContext(nc) as tc, ExitStack() as ctx:
        wpool = ctx.enter_context(tc.tile_pool(name="weights", bufs=1))
        per_b = ctx.enter_context(tc.tile_pool(name="per_b", bufs=1))
        xpool = ctx.enter_context(tc.tile_pool(name="xstream", bufs=2))
        qkvpool = ctx.enter_context(tc.tile_pool(name="qkv", bufs=4))
        epool = ctx.enter_context(tc.tile_pool(name="exp", bufs=6))
        rpool = ctx.enter_context(tc.tile_pool(name="rope", bufs=2))
        opool = ctx.enter_context(tc.tile_pool(name="out", bufs=2))
        spool = ctx.enter_context(tc.tile_pool(name="small", bufs=4))
        pp_proj = ctx.enter_context(tc.tile_pool(name="pproj", bufs=2, space="PSUM"))
        pp_att = ctx.enter_context(tc.tile_pool(name="patt", bufs=2, space="PSUM"))
        pp_av = ctx.enter_context(tc.tile_pool(name="pav", bufs=2, space="PSUM"))
        pp_o = ctx.enter_context(tc.tile_pool(name="po", bufs=2, space="PSUM"))

        # ---- persistent weights/tables ----
        wall_sb = wpool.tile([128, KC, 384], F16, tag="wall")
        wot_sb = wpool.tile([128, 2, D], F16, tag="wot")
        cos_sb = wpool.tile([128, NJC, 160], F16, tag="cos")
        sin_sb = wpool.tile([128, NJC, 160], F16, tag="sin")
        mask_sb = wpool.tile([128, 4, TT], F16, tag="mask")
        for k in range(KC):
            nc.sync.dma_start(wall_sb[:, k, :], wall[k * 128:(k + 1) * 128, :])
        for cc in range(2):
            nc.sync.dma_start(wot_sb[:, cc, :], wot[cc * 128:(cc + 1) * 128, :])
        for j in range(NJC):
            nc.sync.dma_start(cos_sb[:, j, :], cosr[j * 128:(j + 1) * 128, :])
            nc.sync.dma_start(sin_sb[:, j, :], sinr[j * 128:(j + 1) * 128, :])
        for m in range(4):
            nc.sync.dma_start(mask_sb[:, m, :], cmask[m])

        # per-batch persistent tiles
        qt = {}     # [128, 2, S]: pair p holds heads 2p (part 0:64), 2p+1 (64:128)
        ktv = {}    # [128, S]: rows 0:64 = K^T; rows 64:128 = V^T (unused)
        ktv2 = {}   # [128, S]: rows 64:128 = K^T copy (for odd heads)
        vsb = {}    # [128, NJC, 65]: V natural [tok, dh] + ones column
        at = {}     # [128, 2, S]: o-proj stationary (channels x tokens)
        nrm = {}    # [128, 2, NJC, 128]: normalized attn [q, dh] pair-packed
        for b in range(B):
            qt[b] = per_b.tile([128, 2, S], F16, tag=f"qt{b}", name=f"qt{b}")
            ktv[b] = per_b.tile([128, S], F16, tag=f"ktv{b}", name=f"ktv{b}")
            ktv2[b] = per_b.tile([128, S], F16, tag=f"ktv2{b}", name=f"ktv2{b}")
            vsb[b] = per_b.tile([128, NJC, 65], F16, tag=f"vsb{b}", name=f"vsb{b}")
            at[b] = per_b.tile([128, 2, S], F16, tag=f"at{b}", name=f"at{b}")
            nrm[b] = per_b.tile([128, 2, NJC, 128], F16, tag=f"nrm{b}",
                                name=f"nrm{b}")
            nc.vector.memset(vsb[b][:, :, 64:65], 1.0)

        # ---------- QKV projection + rope (emitted as interleavable units) ----
        def qkv_units(b):
            """Closures, each emitting one 128-token tile's projection work:
            16 PE matmuls + DVE evict/rope + SP transposes."""
            units = []
            xts = {}

            def make_unit(tt):
                def emit():
                    tg = tt // 4
                    if tt % 4 == 0:
                        xts[tg] = xpool.tile([128, KC, TT], F16, tag="xt",
                                             name=f"xt_{b}_{tg}")
                        nc.sync.dma_start(
                            xts[tg][:],
                            xt[b].rearrange("(kc p) s -> p kc s", p=128)
                            [:, :, tg * TT:(tg + 1) * TT])
                    xtile = xts[tg]
                    s0 = (tt % 4) * 128
                    pq = pp_proj.tile([128, 384], F32, tag="pq",
                                      name=f"pq_{b}_{tt}")
                    for k in range(KC):
                        nc.tensor.matmul(pq[:], xtile[:, k, s0:s0 + 128],
                                         wall_sb[:, k, :],
                                         start=(k == 0), stop=(k == KC - 1))
                    qkv = qkvpool.tile([128, 384], F16, tag="qkv")
                    nc.vector.tensor_copy(qkv[:], pq[:])
                    # rope on q+k (cols 0:320); host permuted each head's dims
                    # to [32 evens | 32 odds] so these slices are stride-1
                    pear = qkv[:, 0:320].rearrange("p (h half i) -> p h half i",
                                                   half=2, i=32)
                    ev, od = pear[:, :, 0, :], pear[:, :, 1, :]
                    cs = cos_sb[:, tt, :].rearrange("p (h i) -> p h i", i=32)
                    sn = sin_sb[:, tt, :].rearrange("p (h i) -> p h i", i=32)
                    ec = rpool.tile([128, 5, 32], F16, tag="ec")
                    es = rpool.tile([128, 5, 32], F16, tag="es")
                    oc = rpool.tile([128, 5, 32], F16, tag="oc")
                    os_ = rpool.tile([128, 5, 32], F16, tag="os")
                    nc.vector.tensor_mul(ec[:], ev, cs)
                    nc.vector.tensor_mul(es[:], ev, sn)
                    nc.vector.tensor_mul(oc[:], od, cs)
                    nc.vector.tensor_mul(os_[:], od, sn)
                    nc.vector.tensor_sub(ev, ec[:], os_[:])
                    nc.vector.tensor_add(od, es[:], oc[:])
                    nc.vector.tensor_copy(vsb[b][:, tt, 0:64], qkv[:, 320:384])
                    tsl = slice(tt * 128, (tt + 1) * 128)
                    nc.sync.dma_start_transpose(qt[b][:, 0, tsl], qkv[:, 0:128])
                    nc.sync.dma_start_transpose(qt[b][:, 1, tsl], qkv[:, 128:256])
                    nc.sync.dma_start_transpose(ktv[b][:, tsl], qkv[:, 256:384])
                return emit

            for tt in range(NJC):
                units.append(make_unit(tt))
            return units

        def emit_ktv2(b):
            # K^T replicated to partitions 64:127 (DMA moves across partitions)
            nc.sync.dma_start(ktv2[b][64:128, :], ktv[b][0:64, :])

        # ---------- attention (q-outer, ACT-paced; drains bg PE units) ------
        def emit_attn(b, bg):
            n_chunks = G * sum(4 * it + 4 for it in range(NTT))
            per = len(bg) / n_chunks if bg else 0.0
            acc = 0.0
            done = 0.0
            for g in range(G):
                base, pair, cc = (g % 2) * 64, g // 2, g // 2
                kst = ktv[b] if g % 2 == 0 else ktv2[b]
                for it in range(NTT):
                    isl = slice(it * TT, (it + 1) * TT)
                    av = pp_av.tile([128, 4, 128], F32, tag="av",
                                    name=f"av_{b}_{g}_{it}")
                    njc = 4 * it + 4
                    pending = []
                    for jc in range(njc):
                        psc = pp_att.tile([128, TT], F32, tag="sc")
                        nc.tensor.matmul(
                            psc[:], kst[base:base + 64, jc * 128:(jc + 1) * 128],
                            qt[b][base:base + 64, pair, isl],
                            start=True, stop=True)
                        esb = epool.tile([128, TT], F16, tag="exp")
                        nc.scalar.activation(esb[:], psc[:], EXP, scale=0.125)
                        if jc >= 4 * it:  # diagonal block: causal mask
                            nc.vector.tensor_mul(esb[:], esb[:],
                                                 mask_sb[:, jc - 4 * it, :])
                        pending.append((jc, esb))
                        if len(pending) > 2:
                            jd, ed = pending.pop(0)
                            for sub in range(4):
                                nc.tensor.matmul(
                                    av[:, sub, 0:65],
                                    ed[:, sub * 128:(sub + 1) * 128],
                                    vsb[b][:, jd, :],
                                    start=(jd == 0), stop=(jd == njc - 1),
                                    skip_group_check=True)
                        acc += per
                        while done < acc and bg:
                            bg.popleft()()
                            done += 1.0
                    for jd, ed in pending:
                        for sub in range(4):
                            nc.tensor.matmul(
                                av[:, sub, 0:65],
                                ed[:, sub * 128:(sub + 1) * 128],
                                vsb[b][:, jd, :],
                                start=(jd == 0), stop=(jd == njc - 1),
                                skip_group_check=True)
                    # normalize: recip of ones-column sum, per-partition scale
                    for sub in range(4):
                        qc = it * 4 + sub
                        rcp = spool.tile([128, 1], F32, tag="rcp")
                        nc.vector.reciprocal(rcp[:], av[:, sub, 64:65])
                        nc.vector.tensor_scalar_mul(
                            nrm[b][:, cc, qc, base:base + 64],
                            av[:, sub, 0:64], rcp[:])
                if g % 2 == 1:  # pair done: transpose into o-proj layout
                    for qc in range(NJC):
                        nc.sync.dma_start_transpose(
                            at[b][:, cc, qc * 128:(qc + 1) * 128],
                            nrm[b][:, cc, qc, :])
            while bg:
                bg.popleft()()

        # ---------- o projection (partial over this core's 256 channels) ----
        def oproj_units(b):
            units = []

            def make_unit(tt):
                def emit():
                    tsl = slice(tt * 128, (tt + 1) * 128)
                    ob = opool.tile([128, 4, TT], F16, tag="osb")
                    for nt in range(4):
                        nsl = slice(nt * TT, (nt + 1) * TT)
                        po = pp_o.tile([128, TT], F32, tag="po",
                                       name=f"po_{b}_{tt}_{nt}")
                        nc.tensor.matmul(po[:], at[b][:, 0, tsl],
                                         wot_sb[:, 0, nsl], start=True, stop=False)
                        nc.tensor.matmul(po[:], at[b][:, 1, tsl],
                                         wot_sb[:, 1, nsl], start=False, stop=True)
                        nc.gpsimd.tensor_copy(ob[:, nt, :], po[:])
                    nc.gpsimd.dma_start(o[b, tsl, :], ob[:])
                return emit

            for tt in range(NJC):
                units.append(make_unit(tt))
            return units

        # ---------- schedule ----------
        for u in qkv_units(0):
            u()
        emit_ktv2(0)
        emit_attn(0, deque(qkv_units(1)))
        emit_ktv2(1)
        emit_attn(1, deque(oproj_units(0)))
        for u in oproj_units(1):
            u()
    nc.compile()
    return nc


def host_inputs(x, Wq, Wk, Wv, Wo):
    """Per-core input maps. Q/K weight rows permuted per head to
    [32 even dims | 32 odd dims] so device rope slices are stride-1."""
    xtp = np.ascontiguousarray(
        np.transpose(np.asarray(x, np.float32), (0, 2, 1))).astype(np.float16)
    inv = ROPE_BASE ** (-np.arange(0, DH, 2, dtype=np.float64) / DH)
    th = np.arange(S, dtype=np.float64)[:, None] * inv[None, :]  # (S, 32)
    cosr = np.tile(np.cos(th), (1, 5)).astype(np.float16)  # (S, 160)
    sinr = np.tile(np.sin(th), (1, 5)).astype(np.float16)
    p = np.arange(128)[:, None]
    f = np.arange(TT)[None, :]
    cmask = np.stack([(p + m * 128 <= f).astype(np.float16) for m in range(4)])
    perm = np.concatenate([np.arange(0, DH, 2), np.arange(1, DH, 2)])
    in_maps = []
    for c in range(NCORES):
        qrows = [Wq[(4 * c + h) * DH:(4 * c + h + 1) * DH][perm] for h in range(G)]
        krows = Wk[DH * c:DH * (c + 1)][perm]
        vrows = Wv[DH * c:DH * (c + 1)]
        wall = np.concatenate(qrows + [krows, vrows], axis=0)     # (384, D)
        wall = np.ascontiguousarray(wall.T.astype(np.float16))    # (D, 384)
        wot = np.ascontiguousarray(Wo[:, 256 * c:256 * (c + 1)].T
                                   .astype(np.float16))           # (256, D)
        in_maps.append(dict(xt=xtp, wall=wall, wot=wot, cosr=cosr, sinr=sinr,
                            cmask=cmask))
    return in_maps


def kernel(**inputs):
    x = np.asarray(inputs["x"], dtype=np.float32)
    Wq = np.asarray(inputs["Wq"], dtype=np.float32)
    Wk = np.asarray(inputs["Wk"], dtype=np.float32)
    Wv = np.asarray(inputs["Wv"], dtype=np.float32)
    Wo = np.asarray(inputs["Wo"], dtype=np.float32)
    in_maps = host_inputs(x, Wq, Wk, Wv, Wo)
    if "nc" not in _cached:
        _cached["nc"] = build_nc()
    res = run_bass_kernel_spmd(_cached["nc"], in_maps, list(range(NCORES)))
    out = np.zeros((B, S, D), np.float64)
    for r in res.results:
        out += np.asarray(r["o"], np.float64)
    return out.astype(np.float32)


# revision 10
# speedup vs baseline: 1.2229x; 1.0651x over previous
"""GQA (32 q heads / 8 kv heads, RoPE, causal) Trainium2 Bass kernel.

Sharding: tensor-parallel over kv heads -- core c owns kv head c and q heads
4c..4c+3 for both batches. Each core computes a partial o-projection
(its 256 attn channels x Wo columns) in fp16 and the host sums the 8 partials.

Device-side structure (per core, per batch):
  * QKV projection in fp16: stationary x chunks [d,128 tok], moving fused
    W [d, 384] = [4 q heads | k | v]; psum evicted to fp16 by DVE.
  * RoPE on DVE in token-partition layout; head dims are host-permuted to
    [32 evens | 32 odds] so the pair slices are stride-1 (DVE fast mode).
  * Q/K transposed to [dh, tok] via DMA-XBAR transposes (no PE involvement);
    K replicated to partitions 64:127 with one SBUF-to-SBUF DMA per batch so
    odd heads can use matching base partitions.
  * Scores computed [keys, q] per 128-key chunk; exp on ACT (scale=1/8, no
    max needed); causal diagonal masked by DVE multiply in fp16.
  * attn.V restructured: out [q, dh+1] psum (65 rows per 128-key chunk, half
    the PE cost of the [dh, q] form); ones column of V gives the softmax
    denominator; normalization = DVE reciprocal + per-partition scalar mul.
  * Normalized attn tiles [q, dh] are pair-packed and DMA-XBAR-transposed
    into the o-projection stationary layout [channels, tok].
  * o-projection psum evicted to fp16 by GPSIMD; o written back by
    GPSIMD-issued SWDGE DMAs (keeps the shared HWDGE free).
  * QKV matmuls of batch 1 and o-proj matmuls of batch 0 are interleaved
    into the ACT-bound attention phases so the PE never starves.
"""

import numpy as np
from collections import deque
from contextlib import ExitStack

import concourse.bass as bass
from concourse import bacc
import concourse.mybir as mybir
import concourse.tile as tile
from concourse.bass_utils import run_bass_kernel_spmd

B, S, D = 2, 2048, 2048
DH = 64            # head dim
G = 4              # q heads per core (= per kv head)
NCORES = 8
TT = 512           # attention i-tile
NTT = S // TT      # 4
KC = D // 128      # 16 contraction chunks
NJC = S // 128     # 16 token/key chunks of 128
F32 = mybir.dt.float32
F16 = mybir.dt.float16
ROPE_BASE = 10000.0

_cached = {}


def build_nc():
    nc = bacc.Bacc("TRN2", target_bir_lowering=False, debug=False)
    xt = nc.declare_dram_parameter("xt", [B, D, S], F16, isOutput=False)
    wall = nc.declare_dram_parameter("wall", [D, 384], F16, isOutput=False)
    wot = nc.declare_dram_parameter("wot", [256, D], F16, isOutput=False)
    cosr = nc.declare_dram_parameter("cosr", [S, 160], F16, isOutput=False)
    sinr = nc.declare_dram_parameter("sinr", [S, 160], F16, isOutput=False)
    cmask = nc.declare_dram_parameter("cmask", [4, 128, TT], F16, isOutput=False)
    o = nc.declare_dram_parameter("o", [B, S, D], F16, isOutput=True)

    EXP = mybir.ActivationFunctionType.Exp

    with tile.TileContext(nc) as tc, ExitStack() as ctx:
        wpool = ctx.enter_context(tc.tile_pool(name="weights", bufs=1))
        per_b = ctx.enter_context(tc.tile_pool(name="per_b", bufs=1))
        xpool = ctx.enter_context(tc.tile_pool(name="xstream", bufs=2))
        qkvpool = ctx.enter_context(tc.tile_pool(name="qkv", bufs=4))
        epool = ctx.enter_context(tc.tile_pool(name="exp", bufs=6))
        rpool = ctx.enter_context(tc.tile_pool(name="rope", bufs=2))
        opool = ctx.enter_context(tc.tile_pool(name="out", bufs=2))
        spool = ctx.enter_context(tc.tile_pool(name="small", bufs=4))
        pp_proj = ctx.enter_context(tc.tile_pool(name="pproj", bufs=2, space="PSUM"))
        pp_att = ctx.enter_context(tc.tile_pool(name="patt", bufs=3, space="PSUM"))
        pp_av = ctx.enter_context(tc.tile_pool(name="pav", bufs=1, space="PSUM"))
        pp_o = ctx.enter_context(tc.tile_pool(name="po", bufs=2, space="PSUM"))

        # ---- persistent weights/tables ----
        wall_sb = wpool.tile([128, KC, 384], F16, tag="wall")
        wot_sb = wpool.tile([128, 2, D], F16, tag="wot")
        cos_sb = wpool.tile([128, NJC, 160], F16, tag="cos")
        sin_sb = wpool.tile([128, NJC, 160], F16, tag="sin")
        mask_sb = wpool.tile([128, 4, TT], F16, tag="mask")

        def load_weights():
            nc.sync.dma_start(wall_sb[:],
                              wall.rearrange("(kc p) n -> p kc n", p=128))
            nc.sync.dma_start(wot_sb[:],
                              wot.rearrange("(cc p) n -> p cc n", p=128))
            nc.sync.dma_start(cos_sb[:],
                              cosr.rearrange("(j p) n -> p j n", p=128))
            nc.sync.dma_start(sin_sb[:],
                              sinr.rearrange("(j p) n -> p j n", p=128))
            nc.sync.dma_start(mask_sb[:], cmask.rearrange("m p t -> p m t"))

        # per-batch persistent tiles
        qt = {}     # [128, 2, S]: pair p holds heads 2p (part 0:64), 2p+1 (64:128)
        ktv = {}    # [128, S]: rows 0:64 = K^T; rows 64:128 = V^T (unused)
        ktv2 = {}   # [128, S]: rows 64:128 = K^T copy (for odd heads)
        vsb = {}    # [128, NJC, 65]: V natural [tok, dh] + ones column
        at = {}     # [128, 2, S]: o-proj stationary (channels x tokens)
        nrm = {}    # [128, 2, NJC, 128]: normalized attn [q, dh] pair-packed
        for b in range(B):
            qt[b] = per_b.tile([128, 2, S], F16, tag=f"qt{b}", name=f"qt{b}")
            ktv[b] = per_b.tile([128, S], F16, tag=f"ktv{b}", name=f"ktv{b}")
            ktv2[b] = per_b.tile([128, S], F16, tag=f"ktv2{b}", name=f"ktv2{b}")
            vsb[b] = per_b.tile([128, NJC, 65], F16, tag=f"vsb{b}", name=f"vsb{b}")
            at[b] = per_b.tile([128, 2, S], F16, tag=f"at{b}", name=f"at{b}")
            nrm[b] = per_b.tile([128, 2, NJC, 128], F16, tag=f"nrm{b}",
                                name=f"nrm{b}")
            nc.vector.memset(vsb[b][:, :, 64:65], 1.0)

        # ---------- QKV projection + rope (emitted as interleavable units) ----
        def qkv_units(b):
            """Closures, each emitting a quarter-tile of projection work
            (4 PE matmuls); the last quarter adds DVE evict/rope + SP
            transposes. Fine granularity lets attention interleave them."""
            units = []
            xts = {}
            pqs = {}

            def load_x(tg):
                def emit():
                    xts[tg] = xpool.tile([128, KC, TT], F16, tag="xt",
                                         name=f"xt_{b}_{tg}")
                    nc.sync.dma_start(
                        xts[tg][:],
                        xt[b].rearrange("(kc p) s -> p kc s", p=128)
                        [:, :, tg * TT:(tg + 1) * TT])
                return emit

            def make_unit(tt, quarter):
                def emit():
                    tg = tt // 4
                    xtile = xts[tg]
                    s0 = (tt % 4) * 128
                    if quarter == 0:
                        pqs[tt] = pp_proj.tile([128, 384], F32, tag="pq",
                                               name=f"pq_{b}_{tt}")
                    pq = pqs[tt]
                    for k in range(quarter * 4, quarter * 4 + 4):
                        nc.tensor.matmul(pq[:], xtile[:, k, s0:s0 + 128],
                                         wall_sb[:, k, :],
                                         start=(k == 0), stop=(k == KC - 1))
                    if quarter < 3:
                        return
                    del pqs[tt]
                    qkv = qkvpool.tile([128, 384], F16, tag="qkv")
                    nc.vector.tensor_copy(qkv[:], pq[:])
                    # rope on q+k (cols 0:320); host permuted each head's dims
                    # to [32 evens | 32 odds] so these slices are stride-1
                    pear = qkv[:, 0:320].rearrange("p (h half i) -> p h half i",
                                                   half=2, i=32)
                    ev, od = pear[:, :, 0, :], pear[:, :, 1, :]
                    cs = cos_sb[:, tt, :].rearrange("p (h i) -> p h i", i=32)
                    sn = sin_sb[:, tt, :].rearrange("p (h i) -> p h i", i=32)
                    ec = rpool.tile([128, 5, 32], F16, tag="ec")
                    es = rpool.tile([128, 5, 32], F16, tag="es")
                    oc = rpool.tile([128, 5, 32], F16, tag="oc")
                    os_ = rpool.tile([128, 5, 32], F16, tag="os")
                    nc.vector.tensor_mul(ec[:], ev, cs)
                    nc.vector.tensor_mul(es[:], ev, sn)
                    nc.vector.tensor_mul(oc[:], od, cs)
                    nc.vector.tensor_mul(os_[:], od, sn)
                    nc.vector.tensor_sub(ev, ec[:], os_[:])
                    nc.vector.tensor_add(od, es[:], oc[:])
                    nc.vector.tensor_copy(vsb[b][:, tt, 0:64], qkv[:, 320:384])
                    tsl = slice(tt * 128, (tt + 1) * 128)
                    nc.sync.dma_start_transpose(qt[b][:, 0, tsl], qkv[:, 0:128])
                    nc.sync.dma_start_transpose(qt[b][:, 1, tsl], qkv[:, 128:256])
                    nc.sync.dma_start_transpose(ktv[b][:, tsl], qkv[:, 256:384])
                return emit

            for tt in range(NJC):
                if tt % 4 == 0:
                    units.append(load_x(tt // 4))
                for quarter in range(4):
                    units.append(make_unit(tt, quarter))
            return units

        def emit_ktv2(b):
            # K^T replicated to partitions 64:127 (DMA moves across partitions)
            nc.sync.dma_start(ktv2[b][64:128, :], ktv[b][0:64, :])

        # ---------- attention (q-outer, ACT-paced; drains bg PE units) ------
        def emit_attn(b, bg):
            n_chunks = G * sum(4 * it + 4 for it in range(NTT))
            per = len(bg) / n_chunks if bg else 0.0
            acc = 0.0
            done = 0.0
            for g in range(G):
                base, pair, cc = (g % 2) * 64, g // 2, g // 2
                kst = ktv[b] if g % 2 == 0 else ktv2[b]
                for it in range(NTT):
                    isl = slice(it * TT, (it + 1) * TT)
                    av = pp_av.tile([128, 4, 128], F32, tag="av",
                                    name=f"av_{b}_{g}_{it}")
                    njc = 4 * it + 4
                    pending = []
                    for jc in range(njc):
                        psc = pp_att.tile([128, TT], F32, tag="sc")
                        nc.tensor.matmul(
                            psc[:], kst[base:base + 64, jc * 128:(jc + 1) * 128],
                            qt[b][base:base + 64, pair, isl],
                            start=True, stop=True)
                        esb = epool.tile([128, TT], F16, tag="exp")
                        nc.scalar.activation(esb[:], psc[:], EXP, scale=0.125)
                        if jc >= 4 * it:  # diagonal block: causal mask
                            nc.vector.tensor_mul(esb[:], esb[:],
                                                 mask_sb[:, jc - 4 * it, :])
                        pending.append((jc, esb))
                        if len(pending) > 2:
                            jd, ed = pending.pop(0)
                            for sub in range(4):
                                nc.tensor.matmul(
                                    av[:, sub, 0:65],
                                    ed[:, sub * 128:(sub + 1) * 128],
                                    vsb[b][:, jd, :],
                                    start=(jd == 0), stop=(jd == njc - 1),
                                    skip_group_check=True)
                        acc += per
                        while done < acc and bg:
                            bg.popleft()()
                            done += 1.0
                    for jd, ed in pending:
                        for sub in range(4):
                            nc.tensor.matmul(
                                av[:, sub, 0:65],
                                ed[:, sub * 128:(sub + 1) * 128],
                                vsb[b][:, jd, :],
                                start=(jd == 0), stop=(jd == njc - 1),
                                skip_group_check=True)
                    # normalize: recip of ones-column sum, per-partition scale
                    for sub in range(4):
                        qc = it * 4 + sub
                        rcp = spool.tile([128, 1], F32, tag="rcp")
                        nc.vector.reciprocal(rcp[:], av[:, sub, 64:65])
                        nc.vector.tensor_scalar_mul(
                            nrm[b][:, cc, qc, base:base + 64],
                            av[:, sub, 0:64], rcp[:])
                if g % 2 == 1:  # pair done: transpose into o-proj layout
                    for qc in range(NJC):
                        nc.sync.dma_start_transpose(
                            at[b][:, cc, qc * 128:(qc + 1) * 128],
                            nrm[b][:, cc, qc, :])
            while bg:
                bg.popleft()()

        # ---------- o projection (partial over this core's 256 channels) ----
        def oproj_units(b):
            units = []
            obs = {}

            def make_unit(tt, nt):
                def emit():
                    tsl = slice(tt * 128, (tt + 1) * 128)
                    if nt == 0:
                        obs[tt] = opool.tile([128, 4, TT], F16, tag="osb",
                                             name=f"osb_{b}_{tt}")
                    ob = obs[tt]
                    nsl = slice(nt * TT, (nt + 1) * TT)
                    po = pp_o.tile([128, TT], F32, tag="po",
                                   name=f"po_{b}_{tt}_{nt}")
                    nc.tensor.matmul(po[:], at[b][:, 0, tsl],
                                     wot_sb[:, 0, nsl], start=True, stop=False)
                    nc.tensor.matmul(po[:], at[b][:, 1, tsl],
                                     wot_sb[:, 1, nsl], start=False, stop=True)
                    nc.gpsimd.tensor_copy(ob[:, nt, :], po[:])
                    if nt == 3:
                        del obs[tt]
                        nc.gpsimd.dma_start(o[b, tsl, :], ob[:])
                return emit

            for tt in range(NJC):
                for nt in range(4):
                    units.append(make_unit(tt, nt))
            return units

        # ---------- schedule ----------
        u0 = qkv_units(0)
        u0[0]()          # first x tile load ahead of the bulk weight loads
        load_weights()
        for u in u0[1:]:
            u()
        emit_ktv2(0)
        emit_attn(0, deque(qkv_units(1)))
        emit_ktv2(1)
        emit_attn(1, deque(oproj_units(0)))
        for u in oproj_units(1):
            u()
    nc.compile()
    return nc


def host_inputs(x, Wq, Wk, Wv, Wo):
    """Per-core input maps. Q/K weight rows permuted per head to
    [32 even dims | 32 odd dims] so device rope slices are stride-1."""
    xtp = np.ascontiguousarray(
        np.transpose(np.asarray(x, np.float32), (0, 2, 1))).astype(np.float16)
    inv = ROPE_BASE ** (-np.arange(0, DH, 2, dtype=np.float64) / DH)
    th = np.arange(S, dtype=np.float64)[:, None] * inv[None, :]  # (S, 32)
    cosr = np.tile(np.cos(th), (1, 5)).astype(np.float16)  # (S, 160)
    sinr = np.tile(np.sin(th), (1, 5)).astype(np.float16)
    p = np.arange(128)[:, None]
    f = np.arange(TT)[None, :]
    cmask = np.stack([(p + m * 128 <= f).astype(np.float16) for m in range(4)])
    perm = np.concatenate([np.arange(0, DH, 2), np.arange(1, DH, 2)])
    in_maps = []
    for c in range(NCORES):
        qrows = [Wq[(4 * c + h) * DH:(4 * c + h + 1) * DH][perm] for h in range(G)]
        krows = Wk[DH * c:DH * (c + 1)][perm]
        vrows = Wv[DH * c:DH * (c + 1)]
        wall = np.concatenate(qrows + [krows, vrows], axis=0)     # (384, D)
        wall = np.ascontiguousarray(wall.T.astype(np.float16))    # (D, 384)
        wot = np.ascontiguousarray(Wo[:, 256 * c:256 * (c + 1)].T
                                   .astype(np.float16))           # (256, D)
        in_maps.append(dict(xt=xtp, wall=wall, wot=wot, cosr=cosr, sinr=sinr,
                            cmask=cmask))
    return in_maps


def kernel(**inputs):
    x = np.asarray(inputs["x"], dtype=np.float32)
    Wq = np.asarray(inputs["Wq"], dtype=np.float32)
    Wk = np.asarray(inputs["Wk"], dtype=np.float32)
    Wv = np.asarray(inputs["Wv"], dtype=np.float32)
    Wo = np.asarray(inputs["Wo"], dtype=np.float32)
    in_maps = host_inputs(x, Wq, Wk, Wv, Wo)
    if "nc" not in _cached:
        _cached["nc"] = build_nc()
    res = run_bass_kernel_spmd(_cached["nc"], in_maps, list(range(NCORES)))
    out = np.zeros((B, S, D), np.float64)
    for r in res.results:
        out += np.asarray(r["o"], np.float64)
    return out.astype(np.float32)
